# revision 3
# baseline (speedup 1.0000x reference)
"""Multi-head causal attention (B=4, S=2048, D=1024, H=16) on 8 TRN2 cores.

Sharding: core c = (batch b = c//2, head-group g = c%2). Each core computes
8 heads of one batch end-to-end: QKV projections, causal flash attention,
and its half of the output projection (row-parallel Wo). Host sums the two
partial outputs per batch (the "all-reduce") and adds nothing else (bias is
split across the two cores on device).

Device dataflow is fully transposed (xT in, outT out) so no on-device
transposes of activations are needed; only V is PE-transposed (bf16).
Matmul dtypes: float32r for x->QT/KT/VT and scores (full PE rate at N>=512,
~1.5e-4 accuracy), bf16 for post-exp P@V and the Wo projection.
"""
import os
import sys
import types

import numpy as np
import ml_dtypes

from concourse import bacc, tile, bass_utils, mybir

B, S, D, H = 4, 2048, 1024, 16
HD = 64            # head dim
G = 2              # head groups (cores per batch)
DG = D // G        # 512 cols per core
NP = DG // 128     # 4 head-pairs per core
NCH = D // 128     # 8 contraction chunks
SB = 512           # q block
NSB = S // SB      # 4 q blocks
NKB = S // 128     # 16 k blocks
SUP = 3            # k-blocks per scores super-block

f32 = mybir.dt.float32
f32r = mybir.dt.float32r
bf16 = mybir.dt.bfloat16

LAST_RESULTS = None
_CACHE = {}


def _install_trace_shim():
    """Register the axon NTFF profile hook if this image's antenv lacks it."""
    if "antenv.axon_hooks" in sys.modules:
        return
    try:
        from trn_agent_boot.trn_boot import _ntff_profile_via_ctypes

        hook = _ntff_profile_via_ctypes("/opt/axon/libaxon_pjrt.so")
        mod = types.ModuleType("antenv.axon_hooks")
        mod.get_axon_ntff_profile_hook = lambda: hook
        mod.set_axon_ntff_profile_hook = lambda h: None
        sys.modules["antenv.axon_hooks"] = mod
        import antenv

        antenv.axon_hooks = mod
    except Exception:
        pass


def _chunks(n, size):
    out = []
    i = 0
    while i < n:
        out.append(list(range(i, min(i + size, n))))
        i += size
    return out


def _build_program():
    nc = bacc.Bacc("TRN2", target_bir_lowering=False, debug=False)

    xT_d = nc.dram_tensor("xT", [D, S], f32r, kind="ExternalInput").ap()
    wq_d = nc.dram_tensor("Wq", [NP, 128, NCH, 128], f32r, kind="ExternalInput").ap()
    wk_d = nc.dram_tensor("Wk", [NP, 128, NCH, 128], f32r, kind="ExternalInput").ap()
    wv_d = nc.dram_tensor("Wv", [NP, 128, NCH, 128], f32r, kind="ExternalInput").ap()
    wo_d = nc.dram_tensor("Wo", [128, NP, NCH, 128], bf16, kind="ExternalInput").ap()
    bo_d = nc.dram_tensor("bo2", [128, NCH], f32, kind="ExternalInput").ap()
    mask_d = nc.dram_tensor("masks", [128, 4, SB], bf16, kind="ExternalInput").ap()
    sel_d = nc.dram_tensor("sel", [33, 128], f32r, kind="ExternalInput").ap()
    id_d = nc.dram_tensor("id128", [128, 128], bf16, kind="ExternalInput").ap()
    out_d = nc.dram_tensor("outT", [D, S], f32, kind="ExternalOutput").ap()

    xT_src = xT_d.rearrange("(c k) s -> k c s", k=128)
    out_dst = out_d.rearrange("(c k) s -> k c s", k=128)

    with tile.TileContext(nc) as tc:
        with (
            tc.tile_pool(name="const", bufs=1) as constp,
            tc.tile_pool(name="ps_big", bufs=2, space="PSUM") as ps_big,
            tc.tile_pool(name="ps_y", bufs=2, space="PSUM") as ps_y,
            tc.tile_pool(name="ynormp", bufs=4) as ynormp,
        ):
            mask_sb = constp.tile([128, 4, SB], bf16)
            sel_sb = constp.tile([33, 128], f32r)
            id_sb = constp.tile([128, 128], bf16)
            bo_sb = constp.tile([128, NCH], f32)
            wo_sb = constp.tile([128, NP, NCH, 128], bf16)
            nc.sync.dma_start(mask_sb[:], mask_d[:])
            nc.sync.dma_start(sel_sb[:], sel_d[:])
            nc.sync.dma_start(id_sb[:], id_d[:])
            nc.sync.dma_start(bo_sb[:], bo_d[:])
            nc.sync.dma_start(wo_sb[:], wo_d[:])

            ynorm = []  # per-pair [128, S] bf16 normalized attention outputs

            with (
                tc.tile_pool(name="xtp", bufs=1) as xtp,
                tc.tile_pool(name="wp", bufs=2) as wp,
                tc.tile_pool(name="qkv", bufs=1) as qkvp,
                tc.tile_pool(name="vtp", bufs=1) as vtp,
                tc.tile_pool(name="vp", bufs=2) as vpool,
                tc.tile_pool(name="pp", bufs=3) as ppool,
                tc.tile_pool(name="yun", bufs=8) as yunp,
                tc.tile_pool(name="den", bufs=1) as denp,
            ):
                xt = xtp.tile([128, NCH, S], f32r)
                nc.sync.dma_start(xt[:], xT_src[:])

                for p in range(NP):
                    # ---- QKV projections for head pair p ----
                    w_tiles = []
                    for nm, wd in (("wq", wq_d), ("wk", wk_d), ("wv", wv_d)):
                        wt = wp.tile([128, NCH, 128], f32r, tag=nm, name=nm)
                        nc.sync.dma_start(wt[:], wd[p])
                        w_tiles.append(wt)

                    qt = qkvp.tile([128, S], f32r, tag="qt", name="qt")
                    kt = qkvp.tile([128, S], f32r, tag="kt", name="kt")
                    vt = vtp.tile([128, S], bf16, name="vt")
                    for wt, dst in zip(w_tiles, (qt, kt, vt)):
                        for grp in _chunks(NSB, SUP):
                            acc = ps_big.tile([128, SUP, SB], f32, tag="big",
                                              name="acc")
                            for ci in range(NCH):
                                for si, sblk in enumerate(grp):
                                    nc.tensor.matmul(
                                        acc[:, si, :],
                                        wt[:, ci, :],
                                        xt[:, ci, sblk * SB:(sblk + 1) * SB],
                                        start=(ci == 0),
                                        stop=(ci == NCH - 1),
                                    )
                            lo, hi = grp[0] * SB, (grp[-1] + 1) * SB
                            nc.scalar.copy(
                                dst[:, lo:hi],
                                acc[:, 0:len(grp), :],
                            )

                    # ---- V: PE-transpose VT -> v_sb [128, kb, head, 65] ----
                    v_sb = vpool.tile([128, NKB, 2, 65], bf16, tag="v", name="v_sb")
                    nc.vector.memset(v_sb[:, :, :, 64:65], 1.0)
                    for kb in range(NKB):
                        tp = ps_y.tile([128, SB], bf16, tag="y", name="tp")
                        nc.tensor.transpose(
                            tp[:, 0:128],
                            vt[:, kb * 128:(kb + 1) * 128],
                            id_sb[:],
                        )
                        nc.vector.tensor_copy(
                            v_sb[:, kb, :, 0:64],
                            tp[:, 0:128].rearrange("k (h d) -> k h d", h=2),
                        )

                    # ---- causal flash attention for the two heads ----
                    denom = denp.tile([33, S], f32, tag="den", name="denom")
                    recip = denp.tile([33, S], f32, tag="rec", name="recip")
                    recip_r = denp.tile([33, S], f32r, tag="recr", name="recip_r")
                    nc.vector.memset(denom[:], 1.0)
                    y_tiles = {}
                    for h in range(2):
                        hlo, hhi = h * 64, (h + 1) * 64
                        for j in range(NSB):
                            yacc = ps_y.tile([128, SB], f32, tag="y", name="yacc")
                            nkb_j = 4 * (j + 1)
                            supers = _chunks(nkb_j, SUP)
                            for sup in supers:
                                sc = ps_big.tile([128, SUP, SB], f32, tag="big",
                                                 name="sc")
                                for si, kb in enumerate(sup):
                                    nc.tensor.matmul(
                                        sc[:, si, :],
                                        kt[hlo:hhi, kb * 128:(kb + 1) * 128],
                                        qt[hlo:hhi, j * SB:(j + 1) * SB],
                                        start=True,
                                        stop=True,
                                    )
                                pt = ppool.tile([128, SUP, SB], bf16, tag="p",
                                                name="pt")
                                ns = len(sup)
                                nc.scalar.activation(
                                    pt[:, 0:ns, :],
                                    sc[:, 0:ns, :],
                                    mybir.ActivationFunctionType.Exp,
                                    scale=0.125,
                                )
                                for si, kb in enumerate(sup):
                                    d = kb - 4 * j
                                    if d >= 0:  # diagonal block: causal mask
                                        nc.vector.tensor_mul(
                                            pt[:, si, :],
                                            pt[:, si, :],
                                            mask_sb[:, d, :],
                                        )
                                for si, kb in enumerate(sup):
                                    nc.tensor.matmul(
                                        yacc[0:65, :],
                                        v_sb[:, kb, h, :],
                                        pt[:, si, :],
                                        start=(kb == 0),
                                        stop=(kb == nkb_j - 1),
                                    )
                            y_un = yunp.tile([64, SB], f32, tag="yun", name="y_un")
                            nc.vector.tensor_copy(y_un[:], yacc[0:64, :])
                            nc.vector.tensor_copy(
                                denom[32 * h:32 * h + 1, j * SB:(j + 1) * SB],
                                yacc[64:65, :],
                            )
                            y_tiles[(h, j)] = y_un

                    # ---- normalize -> ynorm[p] ----
                    yn = ynormp.tile([128, S], bf16, tag="yn", name="yn")
                    nc.vector.reciprocal_approx_fast(recip[:], denom[:])
                    nc.vector.tensor_copy(recip_r[:], recip[:])
                    for j in range(NSB):
                        bc = ps_y.tile([128, SB], f32, tag="y", name="bc")
                        nc.tensor.matmul(
                            bc[:],
                            sel_sb[:],
                            recip_r[:, j * SB:(j + 1) * SB],
                            start=True,
                            stop=True,
                        )
                        for h in range(2):
                            hlo, hhi = h * 64, (h + 1) * 64
                            nc.vector.tensor_mul(
                                yn[hlo:hhi, j * SB:(j + 1) * SB],
                                y_tiles[(h, j)][:],
                                bc[hlo:hhi, :],
                            )
                    ynorm.append(yn)

            # ---- output projection: outT = Wo_g.T @ ynorm (+ bo/2) ----
            with tc.tile_pool(name="outp", bufs=3) as outp:
                for j in range(NSB):
                    for grp in _chunks(NCH, SUP):
                        acc = ps_big.tile([128, SUP, SB], f32, tag="big",
                                          name="oacc")
                        for p in range(NP):
                            for si, dc in enumerate(grp):
                                nc.tensor.matmul(
                                    acc[:, si, :],
                                    wo_sb[:, p, dc, :],
                                    ynorm[p][:, j * SB:(j + 1) * SB],
                                    start=(p == 0),
                                    stop=(p == NP - 1),
                                )
                        ot = outp.tile([128, SUP, SB], f32, tag="ot", name="ot")
                        for si, dc in enumerate(grp):
                            nc.scalar.activation(
                                ot[:, si, :],
                                acc[:, si, :],
                                mybir.ActivationFunctionType.Identity,
                                bias=bo_sb[:, dc:dc + 1],
                            )
                        nc.sync.dma_start(
                            out_dst[:, grp[0]:grp[-1] + 1,
                                    j * SB:(j + 1) * SB],
                            ot[:, 0:len(grp), :],
                        )

    nc.compile()
    return nc


def _get_program():
    if "nc" not in _CACHE:
        _CACHE["nc"] = _build_program()
    return _CACHE["nc"]


def kernel(x, Wq, Wk, Wv, Wo, bo):
    global LAST_RESULTS
    x = np.ascontiguousarray(np.asarray(x, dtype=np.float32))
    Wq = np.asarray(Wq, dtype=np.float32)
    Wk = np.asarray(Wk, dtype=np.float32)
    Wv = np.asarray(Wv, dtype=np.float32)
    Wo = np.asarray(Wo, dtype=np.float32)
    bo = np.asarray(bo, dtype=np.float32)

    nc = _get_program()

    # constants shared by all cores
    masks = np.zeros((128, 4, SB), dtype=ml_dtypes.bfloat16)
    kk = np.arange(128)[:, None]
    qq = np.arange(SB)[None, :]
    for d in range(4):
        masks[:, d, :] = (128 * d + kk <= qq).astype(ml_dtypes.bfloat16)
    sel = np.zeros((33, 128), dtype=np.float32)
    sel[0, 0:64] = 1.0
    sel[32, 64:128] = 1.0
    id128 = np.eye(128, dtype=ml_dtypes.bfloat16)
    bo2 = np.ascontiguousarray((bo / 2.0).reshape(NCH, 128).T)

    in_maps = []
    for c in range(8):
        b, g = c // 2, c % 2
        xT = np.ascontiguousarray(x[b].T)
        wq_g = Wq[:, g * DG:(g + 1) * DG]
        wk_g = Wk[:, g * DG:(g + 1) * DG]
        wv_g = Wv[:, g * DG:(g + 1) * DG]
        wo_g = Wo[g * DG:(g + 1) * DG, :]

        def wshape(w):  # [D, DG] -> [NP, 128, NCH, 128]
            return np.ascontiguousarray(
                w.reshape(NCH, 128, NP, 128).transpose(2, 1, 0, 3))

        wo_dev = np.ascontiguousarray(
            wo_g.reshape(NP, 128, NCH, 128).transpose(1, 0, 2, 3)
        ).astype(ml_dtypes.bfloat16)
        in_maps.append({
            "xT": xT,
            "Wq": wshape(wq_g),
            "Wk": wshape(wk_g),
            "Wv": wshape(wv_g),
            "Wo": wo_dev,
            "bo2": bo2,
            "masks": masks,
            "sel": sel,
            "id128": id128,
        })

    trace = bool(os.environ.get("BASS_TRACE"))
    if trace:
        _install_trace_shim()
    res = bass_utils.run_bass_kernel_spmd(
        nc, in_maps, core_ids=list(range(8)), trace=trace)
    LAST_RESULTS = res

    out = np.empty((B, S, D), dtype=np.float32)
    for b in range(B):
        acc = res.results[2 * b]["outT"] + res.results[2 * b + 1]["outT"]
        out[b] = acc.T
    return out


# revision 4
# speedup vs baseline: 1.2571x; 1.2571x over previous
"""Multi-head causal attention (B=4, S=2048, D=1024, H=16) on 8 TRN2 cores.

Sharding: core c = (batch b = c//2, head-group g = c%2). Each core computes
8 heads of one batch end-to-end: QKV projections, causal flash attention,
and its half of the output projection (row-parallel Wo). Host sums the two
partial outputs per batch (the "all-reduce"); bias is added on device,
split half per core.

Device dataflow is fully transposed (xT in, outT out) so no on-device
transposes of activations are needed; V is PE-transposed (bf16). All
matmuls are bf16 (fp32 PSUM accumulation) except a tiny float32r matmul
that broadcasts softmax reciprocals across partitions. Scores for the two
heads of a pair are issued back-to-back into disjoint PE row groups so
they run concurrently (contraction is only 64 deep). The causal structure
skips invalid 128x512 blocks entirely and trims the invalid left columns
of diagonal blocks from the scores/exp/mask/PV chain.
"""
import os
import sys
import types

import numpy as np
import ml_dtypes

from concourse import bacc, tile, bass_utils, mybir

B, S, D, H = 4, 2048, 1024, 16
HD = 64            # head dim
G = 2              # head groups (cores per batch)
DG = D // G        # 512 cols per core
NP = DG // 128     # 4 head-pairs per core
NCH = D // 128     # 8 contraction chunks
SB = 512           # q block
NSB = S // SB      # 4 q blocks
NKB = S // 128     # 16 k blocks

f32 = mybir.dt.float32
f32r = mybir.dt.float32r
bf16 = mybir.dt.bfloat16

LAST_RESULTS = None
_CACHE = {}


def _install_trace_shim():
    """Register the axon NTFF profile hook if this image's antenv lacks it."""
    if "antenv.axon_hooks" in sys.modules:
        return
    try:
        from trn_agent_boot.trn_boot import _ntff_profile_via_ctypes

        hook = _ntff_profile_via_ctypes("/opt/axon/libaxon_pjrt.so")
        mod = types.ModuleType("antenv.axon_hooks")
        mod.get_axon_ntff_profile_hook = lambda: hook
        mod.set_axon_ntff_profile_hook = lambda h: None
        sys.modules["antenv.axon_hooks"] = mod
        import antenv

        antenv.axon_hooks = mod
    except Exception:
        pass


def _build_program():
    nc = bacc.Bacc("TRN2", target_bir_lowering=False, debug=False)

    xT_d = nc.dram_tensor("xT", [D, S], bf16, kind="ExternalInput").ap()
    wq_d = nc.dram_tensor("Wq", [NP, 128, NCH, 128], bf16, kind="ExternalInput").ap()
    wk_d = nc.dram_tensor("Wk", [NP, 128, NCH, 128], bf16, kind="ExternalInput").ap()
    wv_d = nc.dram_tensor("Wv", [NP, 128, NCH, 128], bf16, kind="ExternalInput").ap()
    wo_d = nc.dram_tensor("Wo", [128, NP, NCH, 128], bf16, kind="ExternalInput").ap()
    bo_d = nc.dram_tensor("bo2", [128, NCH], f32, kind="ExternalInput").ap()
    mask_d = nc.dram_tensor("masks", [128, 4, SB], bf16, kind="ExternalInput").ap()
    sel_d = nc.dram_tensor("sel", [33, 128], f32r, kind="ExternalInput").ap()
    id_d = nc.dram_tensor("id128", [128, 128], bf16, kind="ExternalInput").ap()
    out_d = nc.dram_tensor("outT", [D, S], f32, kind="ExternalOutput").ap()

    xT_src = xT_d.rearrange("(c k) s -> k c s", k=128)
    out_dst = out_d.rearrange("(c k) s -> k c s", k=128)

    with tile.TileContext(nc) as tc:
        with (
            tc.tile_pool(name="const", bufs=1) as constp,
            tc.tile_pool(name="ps2", bufs=3, space="PSUM") as ps2,
            tc.tile_pool(name="ps_y", bufs=2, space="PSUM") as ps_y,
            tc.tile_pool(name="ynormp", bufs=4) as ynormp,
        ):
            mask_sb = constp.tile([128, 4, SB], bf16)
            sel_sb = constp.tile([33, 128], f32r)
            id_sb = constp.tile([128, 128], bf16)
            bo_sb = constp.tile([128, NCH], f32)
            wo_sb = constp.tile([128, NP, NCH, 128], bf16)
            nc.sync.dma_start(mask_sb[:], mask_d[:])
            nc.sync.dma_start(sel_sb[:], sel_d[:])
            nc.sync.dma_start(id_sb[:], id_d[:])
            nc.sync.dma_start(bo_sb[:], bo_d[:])
            nc.sync.dma_start(wo_sb[:], wo_d[:])

            ynorm = []  # per-pair [128, S] bf16 normalized attention outputs

            with (
                tc.tile_pool(name="xtp", bufs=1) as xtp,
                tc.tile_pool(name="wp", bufs=2) as wp,
                tc.tile_pool(name="qkv", bufs=2) as qkvp,
                tc.tile_pool(name="vtp", bufs=1) as vtp,
                tc.tile_pool(name="vp", bufs=2) as vpool,
                tc.tile_pool(name="pp", bufs=3) as ppool,
                tc.tile_pool(name="yun", bufs=8) as yunp,
                tc.tile_pool(name="den", bufs=1) as denp,
            ):
                xt = xtp.tile([128, NCH, S], bf16)
                nc.sync.dma_start(xt[:], xT_src[:])

                for p in range(NP):
                    # ---- QKV projections for head pair p ----
                    w_tiles = []
                    for nm, wd in (("wq", wq_d), ("wk", wk_d), ("wv", wv_d)):
                        wt = wp.tile([128, NCH, 128], bf16, tag=nm, name=nm)
                        nc.sync.dma_start(wt[:], wd[p])
                        w_tiles.append(wt)

                    qt = qkvp.tile([128, S], bf16, tag="qt", name="qt")
                    kt = qkvp.tile([128, S], bf16, tag="kt", name="kt")
                    vt = vtp.tile([128, S], bf16, name="vt")
                    for wt, dst in zip(w_tiles, (qt, kt, vt)):
                        for g2 in range(2):
                            acc = ps2.tile([128, 2, SB], f32, tag="big",
                                           name="acc")
                            for ci in range(NCH):
                                for si in range(2):
                                    sblk = 2 * g2 + si
                                    nc.tensor.matmul(
                                        acc[:, si, :],
                                        wt[:, ci, :],
                                        xt[:, ci, sblk * SB:(sblk + 1) * SB],
                                        start=(ci == 0),
                                        stop=(ci == NCH - 1),
                                    )
                            nc.vector.tensor_copy(
                                dst[:, 2 * g2 * SB:2 * (g2 + 1) * SB],
                                acc[:],
                            )

                    # ---- V: PE-transpose VT -> v_sb [128, kb, head, 65] ----
                    v_sb = vpool.tile([128, NKB, 2, 65], bf16, tag="v", name="v_sb")
                    nc.vector.memset(v_sb[:, :, :, 64:65], 1.0)
                    for kb in range(NKB):
                        tp = ps_y.tile([128, SB], bf16, tag="y", name="tp")
                        nc.tensor.transpose(
                            tp[:, 0:128],
                            vt[:, kb * 128:(kb + 1) * 128],
                            id_sb[:],
                        )
                        nc.vector.tensor_copy(
                            v_sb[:, kb, :, 0:64],
                            tp[:, 0:128].rearrange("k (h d) -> k h d", h=2),
                        )

                    # ---- causal flash attention, both heads interleaved ----
                    denom = denp.tile([33, S], f32, tag="den", name="denom")
                    recip = denp.tile([33, S], f32, tag="rec", name="recip")
                    recip_r = denp.tile([33, S], f32r, tag="recr", name="recip_r")
                    nc.vector.memset(denom[:], 1.0)
                    y_tiles = {}
                    for j in range(NSB):
                        nkb_j = 4 * (j + 1)
                        yaccs = []
                        for h in range(2):
                            yacc = ps_y.tile([128, SB], f32, tag="y",
                                             name="yacc")
                            yaccs.append(yacc)
                        for kb in range(nkb_j):
                            d = kb - 4 * j
                            qlo = max(0, d) * 128  # causal column trim
                            sc = ps2.tile([128, 2, SB], f32, tag="big",
                                          name="sc")
                            for h in range(2):
                                hlo, hhi = h * 64, (h + 1) * 64
                                nc.tensor.matmul(
                                    sc[:, h, qlo:],
                                    kt[hlo:hhi, kb * 128:(kb + 1) * 128],
                                    qt[hlo:hhi, j * SB + qlo:(j + 1) * SB],
                                    start=True,
                                    stop=True,
                                )
                            pt = ppool.tile([128, 2, SB], bf16, tag="p",
                                            name="pt")
                            nc.scalar.activation(
                                pt[:, :, qlo:],
                                sc[:, :, qlo:],
                                mybir.ActivationFunctionType.Exp,
                                scale=0.125,
                            )
                            if d >= 0:
                                for h in range(2):
                                    nc.vector.tensor_mul(
                                        pt[:, h, qlo:],
                                        pt[:, h, qlo:],
                                        mask_sb[:, d, qlo:],
                                    )
                            for h in range(2):
                                nc.tensor.matmul(
                                    yaccs[h][0:65, qlo:],
                                    v_sb[:, kb, h, :],
                                    pt[:, h, qlo:],
                                    start=(kb == 0),
                                    stop=(kb == nkb_j - 1),
                                )
                        for h in range(2):
                            y_un = yunp.tile([64, SB], f32, tag="yun",
                                             name="y_un")
                            nc.vector.tensor_copy(y_un[:], yaccs[h][0:64, :])
                            nc.vector.tensor_copy(
                                denom[32 * h:32 * h + 1, j * SB:(j + 1) * SB],
                                yaccs[h][64:65, :],
                            )
                            y_tiles[(h, j)] = y_un

                    # ---- normalize -> ynorm[p] ----
                    yn = ynormp.tile([128, S], bf16, tag="yn", name="yn")
                    nc.vector.reciprocal_approx_fast(recip[:], denom[:])
                    nc.vector.tensor_copy(recip_r[:], recip[:])
                    for j in range(NSB):
                        bc = ps_y.tile([128, SB], f32, tag="y", name="bc")
                        nc.tensor.matmul(
                            bc[:],
                            sel_sb[:],
                            recip_r[:, j * SB:(j + 1) * SB],
                            start=True,
                            stop=True,
                        )
                        for h in range(2):
                            hlo, hhi = h * 64, (h + 1) * 64
                            nc.vector.tensor_mul(
                                yn[hlo:hhi, j * SB:(j + 1) * SB],
                                y_tiles[(h, j)][:],
                                bc[hlo:hhi, :],
                            )
                    ynorm.append(yn)

            # ---- output projection: outT = Wo_g.T @ ynorm (+ bo/2) ----
            with tc.tile_pool(name="outp", bufs=3) as outp:
                for j in range(NSB):
                    for g2 in range(4):
                        acc = ps2.tile([128, 2, SB], f32, tag="big",
                                       name="oacc")
                        for p in range(NP):
                            for si in range(2):
                                dc = 2 * g2 + si
                                nc.tensor.matmul(
                                    acc[:, si, :],
                                    wo_sb[:, p, dc, :],
                                    ynorm[p][:, j * SB:(j + 1) * SB],
                                    start=(p == 0),
                                    stop=(p == NP - 1),
                                )
                        ot = outp.tile([128, 2, SB], f32, tag="ot", name="ot")
                        for si in range(2):
                            dc = 2 * g2 + si
                            nc.vector.tensor_scalar_add(
                                ot[:, si, :],
                                acc[:, si, :],
                                bo_sb[:, dc:dc + 1],
                            )
                        nc.sync.dma_start(
                            out_dst[:, 2 * g2:2 * g2 + 2,
                                    j * SB:(j + 1) * SB],
                            ot[:],
                        )

    nc.compile()
    return nc


def _get_program():
    if "nc" not in _CACHE:
        _CACHE["nc"] = _build_program()
    return _CACHE["nc"]


def kernel(x, Wq, Wk, Wv, Wo, bo):
    global LAST_RESULTS
    x = np.asarray(x, dtype=np.float32)
    Wq = np.asarray(Wq, dtype=np.float32)
    Wk = np.asarray(Wk, dtype=np.float32)
    Wv = np.asarray(Wv, dtype=np.float32)
    Wo = np.asarray(Wo, dtype=np.float32)
    bo = np.asarray(bo, dtype=np.float32)

    nc = _get_program()

    # constants shared by all cores
    masks = np.zeros((128, 4, SB), dtype=ml_dtypes.bfloat16)
    kk = np.arange(128)[:, None]
    qq = np.arange(SB)[None, :]
    for d in range(4):
        masks[:, d, :] = (128 * d + kk <= qq).astype(ml_dtypes.bfloat16)
    sel = np.zeros((33, 128), dtype=np.float32)
    sel[0, 0:64] = 1.0
    sel[32, 64:128] = 1.0
    id128 = np.eye(128, dtype=ml_dtypes.bfloat16)
    bo2 = np.ascontiguousarray((bo / 2.0).reshape(NCH, 128).T)

    def wshape(w):  # [D, DG] -> [NP, 128, NCH, 128] bf16
        return np.ascontiguousarray(
            w.reshape(NCH, 128, NP, 128).transpose(2, 1, 0, 3)
        ).astype(ml_dtypes.bfloat16)

    in_maps = []
    for c in range(8):
        b, g = c // 2, c % 2
        xT = np.ascontiguousarray(x[b].T).astype(ml_dtypes.bfloat16)
        wo_g = Wo[g * DG:(g + 1) * DG, :]
        wo_dev = np.ascontiguousarray(
            wo_g.reshape(NP, 128, NCH, 128).transpose(1, 0, 2, 3)
        ).astype(ml_dtypes.bfloat16)
        in_maps.append({
            "xT": xT,
            "Wq": wshape(Wq[:, g * DG:(g + 1) * DG]),
            "Wk": wshape(Wk[:, g * DG:(g + 1) * DG]),
            "Wv": wshape(Wv[:, g * DG:(g + 1) * DG]),
            "Wo": wo_dev,
            "bo2": bo2,
            "masks": masks,
            "sel": sel,
            "id128": id128,
        })

    trace = bool(os.environ.get("BASS_TRACE"))
    if trace:
        _install_trace_shim()
    res = bass_utils.run_bass_kernel_spmd(
        nc, in_maps, core_ids=list(range(8)), trace=trace)
    LAST_RESULTS = res

    out = np.empty((B, S, D), dtype=np.float32)
    for b in range(B):
        acc = res.results[2 * b]["outT"] + res.results[2 * b + 1]["outT"]
        out[b] = acc.T
    return out


# revision 7
# speedup vs baseline: 1.2830x; 1.0206x over previous
"""Multi-head causal attention (B=4, S=2048, D=1024, H=16) on 8 TRN2 cores.

Sharding: core c = (batch b = c//2, head-group g = c%2). Each core computes
8 heads of one batch end-to-end: QKV projections, causal flash attention,
and its half of the output projection (row-parallel Wo). Host sums the two
partial outputs per batch (the "all-reduce"); bias is added on device,
split half per core.

Device dataflow is fully transposed (xT in, outT out) so no on-device
transposes of activations are needed; V is PE-transposed (bf16). All
matmuls are bf16 (fp32 PSUM accumulation) except a tiny float32r matmul
that broadcasts softmax reciprocals across partitions. Scores for the two
heads of a pair are issued back-to-back into disjoint PE row groups so
they run concurrently (contraction is only 64 deep). The causal structure
skips invalid 128x512 blocks entirely and trims the invalid left columns
of diagonal blocks from the scores/exp/mask/PV chain.
"""
import os
import sys
import types

import numpy as np
import ml_dtypes

from concourse import bacc, tile, bass_utils, mybir

B, S, D, H = 4, 2048, 1024, 16
HD = 64            # head dim
G = 2              # head groups (cores per batch)
DG = D // G        # 512 cols per core
NP = DG // 128     # 4 head-pairs per core
NCH = D // 128     # 8 contraction chunks
SB = 512           # q block
NSB = S // SB      # 4 q blocks
NKB = S // 128     # 16 k blocks

f32 = mybir.dt.float32
f32r = mybir.dt.float32r
bf16 = mybir.dt.bfloat16

LAST_RESULTS = None
_CACHE = {}


def _install_trace_shim():
    """Register the axon NTFF profile hook if this image's antenv lacks it."""
    if "antenv.axon_hooks" in sys.modules:
        return
    try:
        from trn_agent_boot.trn_boot import _ntff_profile_via_ctypes

        hook = _ntff_profile_via_ctypes("/opt/axon/libaxon_pjrt.so")
        mod = types.ModuleType("antenv.axon_hooks")
        mod.get_axon_ntff_profile_hook = lambda: hook
        mod.set_axon_ntff_profile_hook = lambda h: None
        sys.modules["antenv.axon_hooks"] = mod
        import antenv

        antenv.axon_hooks = mod
    except Exception:
        pass


def _build_program():
    nc = bacc.Bacc("TRN2", target_bir_lowering=False, debug=False)

    xT_d = nc.dram_tensor("xT", [D, S], bf16, kind="ExternalInput").ap()
    wq_d = nc.dram_tensor("Wq", [NP, 128, NCH, 128], bf16, kind="ExternalInput").ap()
    wk_d = nc.dram_tensor("Wk", [NP, 128, NCH, 128], bf16, kind="ExternalInput").ap()
    wv_d = nc.dram_tensor("Wv", [NP, 128, NCH, 128], bf16, kind="ExternalInput").ap()
    wo_d = nc.dram_tensor("Wo", [128, NP, NCH, 128], bf16, kind="ExternalInput").ap()
    bo_d = nc.dram_tensor("bo2", [128, NCH], f32, kind="ExternalInput").ap()
    mask_d = nc.dram_tensor("masks", [128, 4, SB], bf16, kind="ExternalInput").ap()
    sel_d = nc.dram_tensor("sel", [33, 128], f32r, kind="ExternalInput").ap()
    id_d = nc.dram_tensor("id128", [128, 128], bf16, kind="ExternalInput").ap()
    out_d = nc.dram_tensor("outT", [D, S], f32, kind="ExternalOutput").ap()

    xT_src = xT_d.rearrange("(c k) s -> k c s", k=128)
    out_dst = out_d.rearrange("(c k) s -> k c s", k=128)

    with tile.TileContext(nc) as tc:
        with (
            tc.tile_pool(name="const", bufs=1) as constp,
            tc.tile_pool(name="ps2", bufs=3, space="PSUM") as ps2,
            tc.tile_pool(name="ps_y", bufs=2, space="PSUM") as ps_y,
            tc.tile_pool(name="ynormp", bufs=4) as ynormp,
        ):
            mask_sb = constp.tile([128, 4, SB], bf16)
            sel_sb = constp.tile([33, 128], f32r)
            id_sb = constp.tile([128, 128], bf16)
            bo_sb = constp.tile([128, NCH], f32)
            wo_sb = constp.tile([128, NP, NCH, 128], bf16)
            nc.sync.dma_start(mask_sb[:], mask_d[:])
            nc.sync.dma_start(sel_sb[:], sel_d[:])
            nc.sync.dma_start(id_sb[:], id_d[:])
            nc.sync.dma_start(bo_sb[:], bo_d[:])
            nc.sync.dma_start(wo_sb[:], wo_d[:])

            ynorm = []  # per-pair [128, S] bf16 normalized attention outputs

            with (
                tc.tile_pool(name="xtp", bufs=1) as xtp,
                tc.tile_pool(name="wp", bufs=2) as wp,
                tc.tile_pool(name="qkv", bufs=2) as qkvp,
                tc.tile_pool(name="vtp", bufs=1) as vtp,
                tc.tile_pool(name="vp", bufs=2) as vpool,
                tc.tile_pool(name="pp", bufs=4) as ppool,
                tc.tile_pool(name="yun", bufs=8) as yunp,
                tc.tile_pool(name="den", bufs=1) as denp,
            ):
                xt = xtp.tile([128, NCH, S], bf16)
                for ci in range(NCH):
                    nc.sync.dma_start(xt[:, ci, :], xT_src[:, ci, :])

                for p in range(NP):
                    # ---- QKV projections for head pair p ----
                    w_tiles = []
                    for nm, wd in (("wq", wq_d), ("wk", wk_d), ("wv", wv_d)):
                        wt = wp.tile([128, NCH, 128], bf16, tag=nm, name=nm)
                        nc.sync.dma_start(wt[:], wd[p])
                        w_tiles.append(wt)

                    qt = qkvp.tile([128, S], bf16, tag="qt", name="qt")
                    kt = qkvp.tile([128, S], bf16, tag="kt", name="kt")
                    vt = vtp.tile([128, S], bf16, name="vt")
                    for wt, dst in zip(w_tiles, (qt, kt, vt)):
                        for g2 in range(2):
                            acc = ps2.tile([128, 2, SB], f32, tag="big",
                                           name="acc")
                            for ci in range(NCH):
                                for si in range(2):
                                    sblk = 2 * g2 + si
                                    nc.tensor.matmul(
                                        acc[:, si, :],
                                        wt[:, ci, :],
                                        xt[:, ci, sblk * SB:(sblk + 1) * SB],
                                        start=(ci == 0),
                                        stop=(ci == NCH - 1),
                                    )
                            nc.vector.tensor_copy(
                                dst[:, 2 * g2 * SB:2 * (g2 + 1) * SB],
                                acc[:],
                            )

                    # ---- V: PE-transpose VT -> v_sb [128, kb, head, 65] ----
                    v_sb = vpool.tile([128, NKB, 2, 65], bf16, tag="v", name="v_sb")
                    nc.vector.memset(v_sb[:, :, :, 64:65], 1.0)
                    for kb in range(NKB):
                        tp = ps_y.tile([128, SB], bf16, tag="y", name="tp")
                        nc.tensor.transpose(
                            tp[:, 0:128],
                            vt[:, kb * 128:(kb + 1) * 128],
                            id_sb[:],
                        )
                        nc.vector.tensor_copy(
                            v_sb[:, kb, :, 0:64],
                            tp[:, 0:128].rearrange("k (h d) -> k h d", h=2),
                        )

                    # ---- causal flash attention, both heads interleaved ----
                    denom = denp.tile([33, S], f32, tag="den", name="denom")
                    recip = denp.tile([33, S], f32, tag="rec", name="recip")
                    recip_r = denp.tile([33, S], f32r, tag="recr", name="recip_r")
                    nc.vector.memset(denom[:], 1.0)
                    y_tiles = {}
                    for j in range(NSB):
                        nkb_j = 4 * (j + 1)
                        yaccs = []
                        for h in range(2):
                            yacc = ps_y.tile([128, SB], f32, tag="y",
                                             name="yacc")
                            yaccs.append(yacc)
                        for kb in range(nkb_j):
                            d = kb - 4 * j
                            qlo = max(0, d) * 128  # causal column trim
                            sc = ps2.tile([128, 2, SB], f32, tag="big",
                                          name="sc")
                            for h in range(2):
                                hlo, hhi = h * 64, (h + 1) * 64
                                nc.tensor.matmul(
                                    sc[:, h, qlo:],
                                    kt[hlo:hhi, kb * 128:(kb + 1) * 128],
                                    qt[hlo:hhi, j * SB + qlo:(j + 1) * SB],
                                    start=True,
                                    stop=True,
                                )
                            pt = ppool.tile([128, 2, SB], bf16, tag="p",
                                            name="pt")
                            nc.scalar.activation(
                                pt[:, :, qlo:],
                                sc[:, :, qlo:],
                                mybir.ActivationFunctionType.Exp,
                                scale=0.125,
                            )
                            if d >= 0:
                                for h in range(2):
                                    nc.vector.tensor_mul(
                                        pt[:, h, qlo:],
                                        pt[:, h, qlo:],
                                        mask_sb[:, d, qlo:],
                                    )
                            for h in range(2):
                                nc.tensor.matmul(
                                    yaccs[h][0:65, qlo:],
                                    v_sb[:, kb, h, :],
                                    pt[:, h, qlo:],
                                    start=(kb == 0),
                                    stop=(kb == nkb_j - 1),
                                )
                        for h in range(2):
                            y_un = yunp.tile([64, SB], f32, tag="yun",
                                             name="y_un")
                            nc.vector.tensor_copy(y_un[:], yaccs[h][0:64, :])
                            nc.vector.tensor_copy(
                                denom[32 * h:32 * h + 1, j * SB:(j + 1) * SB],
                                yaccs[h][64:65, :],
                            )
                            y_tiles[(h, j)] = y_un

                    # ---- normalize -> ynorm[p] ----
                    yn = ynormp.tile([128, S], bf16, tag="yn", name="yn")
                    nc.vector.reciprocal_approx_fast(recip[:], denom[:])
                    nc.vector.tensor_copy(recip_r[:], recip[:])
                    for j in range(NSB):
                        bc = ps_y.tile([128, SB], f32, tag="y", name="bc")
                        nc.tensor.matmul(
                            bc[:],
                            sel_sb[:],
                            recip_r[:, j * SB:(j + 1) * SB],
                            start=True,
                            stop=True,
                        )
                        for h in range(2):
                            hlo, hhi = h * 64, (h + 1) * 64
                            nc.vector.tensor_mul(
                                yn[hlo:hhi, j * SB:(j + 1) * SB],
                                y_tiles[(h, j)][:],
                                bc[hlo:hhi, :],
                            )
                    ynorm.append(yn)

            # ---- output projection: outT = Wo_g.T @ ynorm (+ bo/2) ----
            with tc.tile_pool(name="outp", bufs=3) as outp:
                for j in range(NSB):
                    for g2 in range(4):
                        acc = ps2.tile([128, 2, SB], f32, tag="big",
                                       name="oacc")
                        for p in range(NP):
                            for si in range(2):
                                dc = 2 * g2 + si
                                nc.tensor.matmul(
                                    acc[:, si, :],
                                    wo_sb[:, p, dc, :],
                                    ynorm[p][:, j * SB:(j + 1) * SB],
                                    start=(p == 0),
                                    stop=(p == NP - 1),
                                )
                        ot = outp.tile([128, 2, SB], f32, tag="ot", name="ot")
                        for si in range(2):
                            dc = 2 * g2 + si
                            nc.vector.tensor_scalar_add(
                                ot[:, si, :],
                                acc[:, si, :],
                                bo_sb[:, dc:dc + 1],
                            )
                        nc.sync.dma_start(
                            out_dst[:, 2 * g2:2 * g2 + 2,
                                    j * SB:(j + 1) * SB],
                            ot[:],
                        )

    nc.compile()
    return nc


def _get_program():
    if "nc" not in _CACHE:
        _CACHE["nc"] = _build_program()
    return _CACHE["nc"]


def kernel(x, Wq, Wk, Wv, Wo, bo):
    global LAST_RESULTS
    x = np.asarray(x, dtype=np.float32)
    Wq = np.asarray(Wq, dtype=np.float32)
    Wk = np.asarray(Wk, dtype=np.float32)
    Wv = np.asarray(Wv, dtype=np.float32)
    Wo = np.asarray(Wo, dtype=np.float32)
    bo = np.asarray(bo, dtype=np.float32)

    nc = _get_program()

    # constants shared by all cores
    masks = np.zeros((128, 4, SB), dtype=ml_dtypes.bfloat16)
    kk = np.arange(128)[:, None]
    qq = np.arange(SB)[None, :]
    for d in range(4):
        masks[:, d, :] = (128 * d + kk <= qq).astype(ml_dtypes.bfloat16)
    sel = np.zeros((33, 128), dtype=np.float32)
    sel[0, 0:64] = 1.0
    sel[32, 64:128] = 1.0
    id128 = np.eye(128, dtype=ml_dtypes.bfloat16)
    bo2 = np.ascontiguousarray((bo / 2.0).reshape(NCH, 128).T)

    def wshape(w):  # [D, DG] -> [NP, 128, NCH, 128] bf16
        return np.ascontiguousarray(
            w.reshape(NCH, 128, NP, 128).transpose(2, 1, 0, 3)
        ).astype(ml_dtypes.bfloat16)

    in_maps = []
    for c in range(8):
        b, g = c // 2, c % 2
        xT = np.ascontiguousarray(x[b].T).astype(ml_dtypes.bfloat16)
        wo_g = Wo[g * DG:(g + 1) * DG, :]
        wo_dev = np.ascontiguousarray(
            wo_g.reshape(NP, 128, NCH, 128).transpose(1, 0, 2, 3)
        ).astype(ml_dtypes.bfloat16)
        in_maps.append({
            "xT": xT,
            "Wq": wshape(Wq[:, g * DG:(g + 1) * DG]),
            "Wk": wshape(Wk[:, g * DG:(g + 1) * DG]),
            "Wv": wshape(Wv[:, g * DG:(g + 1) * DG]),
            "Wo": wo_dev,
            "bo2": bo2,
            "masks": masks,
            "sel": sel,
            "id128": id128,
        })

    trace = bool(os.environ.get("BASS_TRACE"))
    if trace:
        _install_trace_shim()
    res = bass_utils.run_bass_kernel_spmd(
        nc, in_maps, core_ids=list(range(8)), trace=trace)
    LAST_RESULTS = res

    out = np.empty((B, S, D), dtype=np.float32)
    for b in range(B):
        acc = res.results[2 * b]["outT"] + res.results[2 * b + 1]["outT"]
        out[b] = acc.T
    return out
